# revision 24
# baseline (speedup 1.0000x reference)
"""E3Conv Trainium2 kernel: 8-core SPMD, dst-partitioned edges.

Strategy: sort edges by dst; core i owns nodes [1250i,1250(i+1)) and all edges
into them (no all-reduce needed). Per core: node-MLP replicated, one bf16
gather table for Ai (recip applied per-window at the end), radial MLP +
tensor-product restructured as one K=512 matmul per edge tile, PE-accumulated
one-hot matmul scatter. Engine-balanced: PSUM drains fused into DVE
tensor_tensor ops, copies/activations on ACT, scatter accumulation on PE.
"""
import sys, os
sys.path.insert(0, "/opt/trn_rl_repo")
import numpy as np

import concourse.bass as bass
import concourse.tile as tile
from concourse import bacc, mybir
from concourse import bass_utils
from concourse.masks import make_identity

P = 128
N_NODES, N_EDGES, N_GRAPHS = 10000, 131072, 64
N_CORES, NPC, N_WIN = 8, 1250, 10
MAX_RADIUS, N_BASIS = 4.0, 10
STEP = MAX_RADIUS / (N_BASIS + 1)
VCENters = np.linspace(0.0, MAX_RADIUS, N_BASIS + 2)[1:-1].astype(np.float32)
F32, BF16, F32R, I16 = (mybir.dt.float32, mybir.dt.bfloat16,
                        mybir.dt.float32r, mybir.dt.int16)
AF = mybir.ActivationFunctionType
ALU = mybir.AluOpType
NCH = 79  # node chunks of 128 (79*128 = 10112 >= 10000)
TILE_CH = 4
ET = TILE_CH * P  # 512


def _build_consts(fW4):
    s3 = 3.0 ** 0.5
    W4p = np.zeros((512, 224), np.float32)
    offs = {0: 0, 1: 1024, 2: 1536}
    wbase = {0: 0, 1: 16, 2: 24}
    scale_l = {0: 1.0 / 64, 1: s3 / 64, 2: 1.0 / 64}
    for l, mo in enumerate((16, 8, 4)):
        for u in range(8):
            for v in range(8):
                for wl in range(mo):
                    col = offs[l] + (u * 8 + v) * mo + wl
                    w = wbase[l] + wl
                    W4p[np.arange(64) * 8 + v, w * 8 + u] = fW4[:, col] * scale_l[l]
    Sel = np.zeros((4, 64, 128), np.float32)
    for q in range(4):
        for r in range(128):
            Sel[q, 16 * q + r // 8, r] = 1.0
    # L2 reduction weights -> single [112, 60] pair (chained over tm0/tm1)
    # output cols: 0:16 = l0 (16 w), 16:60 = 44 (l1 24 + l2 20) sh-multiplied
    L2_0 = np.zeros((112, 16), np.float32)
    for r in range(112):
        L2_0[r, r // 8] = 1.0
    L2_x = np.zeros((112, 16), np.float32)
    for r in range(16):
        L2_x[r, 14 + r // 8] = 1.0
    L2_1 = np.zeros((112, 44), np.float32)
    for r in range(112):
        w = 14 + r // 8
        if w < 16:
            pass
        elif w < 24:
            for m in range(3):
                L2_1[r, (w - 16) * 3 + m] = 1.0
        else:
            for m in range(5):
                L2_1[r, 24 + (w - 24) * 5 + m] = 1.0
    Wb = np.concatenate([L2_0, np.zeros((112, 44), np.float32)], 1)  # on tm0
    Wa = np.concatenate([L2_x, L2_1], 1)                             # on tm1
    Selsh = np.zeros((8, 44), np.float32)
    for w in range(8):
        for m in range(3):
            Selsh[m, w * 3 + m] = 1.0
    for w in range(4):
        for m in range(5):
            Selsh[3 + m, 24 + w * 5 + m] = 1.0
    return W4p, Sel, Wb, Wa, Selsh


def _host_prep(inputs):
    pos = np.asarray(inputs["pos"], np.float32)
    A = np.asarray(inputs["A"]).astype(np.int64)
    batch = np.asarray(inputs["batch"]).astype(np.int64)
    esrc = np.asarray(inputs["edge_src"]).astype(np.int64)
    edst = np.asarray(inputs["edge_dst"]).astype(np.int64)
    shifts = np.asarray(inputs["edge_shifts"], np.float32)
    cell = np.asarray(inputs["cell"], np.float32)
    counts = np.bincount(edst, minlength=N_NODES).astype(np.float32)
    recipc = 1.0 / np.maximum(counts, 1.0)
    cpn = cell[batch].reshape(N_NODES, 9)
    order = np.argsort(edst, kind="stable")
    wins_all, W_CH = [], 0
    for ci in range(N_CORES):
        lo = ci * NPC
        m = order[(edst[order] >= lo) & (edst[order] < lo + NPC)]
        wins = []
        for w in range(N_WIN):
            wlo = lo + w * P
            whi = min(lo + (w + 1) * P, lo + NPC)
            wm = m[(edst[m] >= wlo) & (edst[m] < whi)]
            wins.append(wm)
            W_CH = max(W_CH, (len(wm) + P - 1) // P)
        wins_all.append(wins)
    while (N_WIN * W_CH) % TILE_CH:
        W_CH += 1
    C_TOT = N_WIN * W_CH
    E = C_TOT * P
    onehotA = np.zeros((10, N_NODES), np.float32)
    onehotA[A, np.arange(N_NODES)] = 1.0
    per_core = []
    for ci in range(N_CORES):
        idx = np.zeros(E, np.int64)
        valid = np.zeros(E, bool)
        dstloc = np.full(E, -1.0, np.float32)
        for w in range(N_WIN):
            wm = wins_all[ci][w]
            s = w * W_CH * P
            idx[s:s + len(wm)] = wm
            valid[s:s + len(wm)] = True
            dstloc[s:s + len(wm)] = (edst[wm] - ci * NPC - w * P).astype(np.float32)
        src = np.where(valid, esrc[idx], 0)
        dst = np.where(valid, edst[idx], 0)
        sh = np.where(valid[:, None], shifts[idx], np.float32(1.0))
        geom = np.concatenate([pos[src], pos[dst], sh, cpn[src]], 1)  # [E,18]
        geom_pl = np.ascontiguousarray(
            np.transpose(geom.reshape(C_TOT, P, 18), (1, 2, 0)).reshape(P, 18 * C_TOT))

        def wrap(ix):
            wr = ix.astype(np.int16).reshape(-1, 16).T  # [16, E/16]
            return np.ascontiguousarray(np.tile(wr, (8, 1)))
        ohm = (dstloc.reshape(C_TOT, P, 1) ==
               np.arange(P, dtype=np.float32)[None, None, :])
        oh_pl = np.ascontiguousarray(
            np.transpose(ohm, (1, 0, 2)).reshape(P, C_TOT * P))
        rw = np.ones((P, N_WIN), np.float32)
        for w in range(N_WIN):
            g0 = ci * NPC + w * P
            n = min(P, NPC - w * P)
            rw[:n, w] = recipc[g0:g0 + n]
        per_core.append(dict(geom_pl=geom_pl, oh_pl=oh_pl, recip_win=rw,
                             idx_src=wrap(src), idx_dst=wrap(dst)))
    return per_core, onehotA, W_CH, C_TOT, E


def _build_bass(W_CH, C_TOT, E, consts):
    W4p, Sel, Wb, Wa, Selsh = consts
    NT = C_TOT // TILE_CH
    NIW = E // 16
    NIWT = NIW // NT
    nc = bacc.Bacc("TRN2", target_bir_lowering=False, debug=False,
                   num_devices=N_CORES)

    def din(name, shape, dt=F32):
        return nc.dram_tensor(name, shape, dt, kind="ExternalInput").ap()

    geom_d = din("geom_pl", [P, 18 * C_TOT])
    ohm_d = din("oh_pl", [P, C_TOT * P], BF16)
    isrc_d = din("idx_src", [P, NIW], I16)
    idst_d = din("idx_dst", [P, NIW], I16)
    ohA_d = din("onehotA", [10, N_NODES], BF16)
    rw_d = din("recip_win", [P, N_WIN])
    TA_d = din("TA", [10, 64], BF16)
    W2_d = din("fit_W2", [64, 32], BF16)
    W3_d = din("fit_W3", [32, 8], BF16)
    W18_d = din("W18", [128, 108], BF16)
    fW2_d = din("fc_W2p", [64, 64], BF16)
    fW3_d = din("fc_W3p", [64, 64], BF16)
    W4p_d = din("W4p", [128, 4 * 224], BF16)
    Sel_d = din("Sel", [64, 4 * 128], BF16)
    Wb_d = din("Wb", [112, 60], BF16)
    Wa_d = din("Wa", [112, 60], BF16)
    cv_d = din("cvec", [P, 16])
    out_d = nc.dram_tensor("out", [N_WIN * P, 60], F32, kind="ExternalOutput").ap()

    C = C_TOT
    with tile.TileContext(nc) as tc:
        with tc.tile_pool(name="const", bufs=1) as cp, \
             tc.tile_pool(name="sb", bufs=2) as sp, \
             tc.tile_pool(name="sb3", bufs=3) as sp3, \
             tc.tile_pool(name="sb4", bufs=4) as sp4, \
             tc.tile_pool(name="big", bufs=1) as bp, \
             tc.tile_pool(name="ps", bufs=2, space="PSUM") as pp, \
             tc.tile_pool(name="dram", bufs=1, space="DRAM") as dp:
            ident = cp.tile([P, P], F32)
            make_identity(nc, ident[:])
            identb = cp.tile([P, P], BF16)
            nc.vector.tensor_copy(identb[:], ident[:])

            def load_const(dram, shape, dt=F32):
                t = cp.tile(shape, dt, tag=dram.tensor.name)
                nc.sync.dma_start(t[:], dram[:])
                return t
            TA = load_const(TA_d, [10, 64], BF16)
            W2 = load_const(W2_d, [64, 32], BF16)
            W3 = load_const(W3_d, [32, 8], BF16)
            W18 = load_const(W18_d, [128, 108], BF16)
            W2e = load_const(fW2_d, [64, 64], BF16)
            W3e = load_const(fW3_d, [64, 64], BF16)
            W4pt = load_const(W4p_d, [128, 4 * 224], BF16)
            Selt = load_const(Sel_d, [64, 4 * 128], BF16)
            Wbt = load_const(Wb_d, [112, 60], BF16)
            Wat = load_const(Wa_d, [112, 60], BF16)
            cv = load_const(cv_d, [P, 16])
            rw = load_const(rw_d, [P, N_WIN])
            ohA = bp.tile([10, N_NODES], BF16)
            nc.sync.dma_start(ohA[:], ohA_d[:])
            isrc = bp.tile([P, NIW], I16)
            nc.sync.dma_start(isrc[:], isrc_d[:])
            idst = bp.tile([P, NIW], I16)
            nc.sync.dma_start(idst[:], idst_d[:])

            # ---------------- node MLP -> gather table ----------------
            Tsrc = dp.tile([NCH * P, P], BF16)
            Ai_sb = bp.tile([8, NCH * P], F32)
            nc.gpsimd.memset(Ai_sb[:], 0.0)
            for j in range(20):
                s = j * 512
                n = min(512, N_NODES - s)
                h1p = pp.tile([64, 512], F32, space="PSUM", tag="fr")
                nc.tensor.matmul(h1p[:, :n], TA[:], ohA[:, s:s + n],
                                 start=True, stop=True)
                h1 = sp.tile([64, 512], BF16, tag="h1n")
                nc.scalar.activation(h1[:, :n], h1p[:, :n], AF.Silu)
                h2p = pp.tile([32, 512], F32, space="PSUM", tag="fr")
                nc.tensor.matmul(h2p[:, :n], W2[:], h1[:, :n],
                                 start=True, stop=True)
                h2 = sp.tile([32, 512], BF16, tag="h2n")
                nc.scalar.activation(h2[:, :n], h2p[:, :n], AF.Silu)
                aip = pp.tile([8, 512], F32, space="PSUM", tag="fr")
                nc.tensor.matmul(aip[:, :n], W3[:], h2[:, :n],
                                 start=True, stop=True)
                nc.vector.tensor_copy(Ai_sb[:, s:s + n], aip[:, :n])
            for c in range(NCH):
                s = c * P
                tp = pp.tile([P, 8], F32, space="PSUM", tag="fr")
                nc.tensor.transpose(tp[:], Ai_sb[:, s:s + P], ident[0:8, 0:8])
                f16a = sp.tile([P, 8], BF16, tag="f16a")
                nc.scalar.copy(f16a[:], tp[:])
                repa = sp.tile([P, P], BF16, tag="repa")
                nc.vector.tensor_copy(
                    repa[:].rearrange("p (r v) -> p r v", v=8),
                    f16a[:].unsqueeze(1).to_broadcast([P, 16, 8]))
                nc.sync.dma_start(Tsrc[s:s + P, :], repa[:])

            # ---------------- geometry (plane layout, whole E) ----------------
            gm = bp.tile([P, 18 * C], F32)
            nc.sync.dma_start(gm[:], geom_d[:])
            g3 = gm[:].rearrange("p (f c) -> p f c", f=18)
            tmp9 = bp.tile([P, 9 * C], F32)
            nc.vector.tensor_tensor(
                out=tmp9[:].rearrange("p (i j c) -> p i j c", i=3, j=3),
                in0=gm[:, 9 * C:18 * C].rearrange("p (i j c) -> p i j c", i=3, j=3),
                in1=gm[:, 6 * C:9 * C].rearrange("p (i c) -> p i c", i=3)
                    .unsqueeze(2).to_broadcast([P, 3, 3, C]),
                op=ALU.mult)
            sv = bp.tile([P, 3 * C], F32)
            nc.vector.tensor_tensor(out=sv[:], in0=tmp9[:, 0:3 * C],
                                    in1=tmp9[:, 3 * C:6 * C], op=ALU.add)
            nc.vector.tensor_tensor(out=sv[:], in0=sv[:],
                                    in1=tmp9[:, 6 * C:9 * C], op=ALU.add)
            ev = bp.tile([P, 3 * C], F32)
            nc.vector.tensor_tensor(out=ev[:], in0=g3[:, 3:6].rearrange("p f c -> p (f c)"),
                                    in1=g3[:, 0:3].rearrange("p f c -> p (f c)"),
                                    op=ALU.subtract)
            nc.vector.tensor_tensor(out=ev[:], in0=ev[:], in1=sv[:], op=ALU.add)
            sq = bp.tile([P, 3 * C], F32)
            nc.vector.tensor_tensor(out=sq[:], in0=ev[:], in1=ev[:], op=ALU.mult)
            ln2 = bp.tile([P, C], F32)
            nc.vector.tensor_tensor(out=ln2[:], in0=sq[:, 0:C], in1=sq[:, C:2 * C],
                                    op=ALU.add)
            nc.vector.tensor_tensor(out=ln2[:], in0=ln2[:], in1=sq[:, 2 * C:3 * C],
                                    op=ALU.add)
            ln = bp.tile([P, C], F32)
            nc.scalar.activation(ln[:], ln2[:], AF.Sqrt)
            rl = bp.tile([P, C], F32)
            nc.vector.reciprocal(rl[:], ln[:])
            u = bp.tile([P, 3 * C], F32)
            nc.vector.tensor_tensor(
                out=u[:].rearrange("p (f c) -> p f c", f=3),
                in0=ev[:].rearrange("p (f c) -> p f c", f=3),
                in1=rl[:].unsqueeze(1).to_broadcast([P, 3, C]), op=ALU.mult)
            usq = bp.tile([P, 3 * C], F32)
            nc.vector.tensor_tensor(out=usq[:], in0=u[:], in1=u[:], op=ALU.mult)
            # feature planes tile: f-major [basis10 | sh1 3 | sh2 5]
            gf = bp.tile([P, 32 * C], F32)
            nc.gpsimd.memset(gf[:, 18 * C:32 * C], 0.0)
            dt2 = bp.tile([P, 10 * C], F32)
            for b in range(N_BASIS):
                nc.scalar.activation(dt2[:, b * C:(b + 1) * C], ln[:], AF.Square,
                                     bias=cv[:, b:b + 1],
                                     scale=cv[:, 10:11])
            nc.scalar.activation(gf[:, 0:10 * C], dt2[:], AF.Exp,
                                 scale=cv[:, 11:12])
            nc.vector.tensor_copy(gf[:, 10 * C:13 * C], u[:])
            t1 = bp.tile([P, C], F32)
            nc.scalar.mul(t1[:], u[:, 2 * C:3 * C], cv[:, 12:13])       # sqrt15*uz
            nc.vector.tensor_tensor(out=gf[:, 13 * C:14 * C], in0=u[:, 0:C],
                                    in1=t1[:], op=ALU.mult)     # m0
            nc.vector.tensor_tensor(out=gf[:, 16 * C:17 * C], in0=u[:, C:2 * C],
                                    in1=t1[:], op=ALU.mult)     # m3
            nc.scalar.mul(t1[:], u[:, 0:C], cv[:, 12:13])               # sqrt15*ux
            nc.vector.tensor_tensor(out=gf[:, 14 * C:15 * C], in0=u[:, C:2 * C],
                                    in1=t1[:], op=ALU.mult)     # m1
            t2 = bp.tile([P, C], F32)
            nc.vector.tensor_tensor(out=t2[:], in0=usq[:, 0:C],
                                    in1=usq[:, 2 * C:3 * C], op=ALU.add)
            nc.scalar.mul(t2[:], t2[:], cv[:, 13:14])
            t3 = bp.tile([P, C], F32)
            nc.scalar.mul(t3[:], usq[:, C:2 * C], cv[:, 14:15])
            nc.vector.tensor_tensor(out=gf[:, 15 * C:16 * C], in0=t3[:], in1=t2[:],
                                    op=ALU.subtract)            # m2
            nc.vector.tensor_tensor(out=t2[:], in0=usq[:, 2 * C:3 * C],
                                    in1=usq[:, 0:C], op=ALU.subtract)
            nc.scalar.mul(gf[:, 17 * C:18 * C], t2[:], cv[:, 15:16])  # m4
            gfc = gf[:].rearrange("p (f c) -> p c f", f=32)

            # F tiles: rows 60:64 must stay zero (read by the transpose);
            # shs tiles: rows 0:16 stay 1.0 (l0 passthrough in the F multiply)
            Fts, shss = [], []
            for fi in range(2):
                Ft = sp.tile([64, ET], BF16, tag="F", name=f"Ft{fi}")
                nc.gpsimd.memset(Ft[:], 0.0)
                Fts.append(Ft)
            for fi in range(4):
                sht = sp4.tile([60, ET], BF16, tag="shs", name=f"sht{fi}")
                nc.gpsimd.memset(sht[0:16, :], 1.0)
                shss.append(sht)

            # ---------------- edge tiles: 3-stage software pipeline ------
            # stage A(j): gathers + geometry transpose + radial MLP front
            # stage B(j): Sel expansion, aiD-mult, K=512 contraction, aiS-mult
            # stage C(j): L2 reduction, sh-mult, transpose + one-hot scatter
            st = {}      # per-tile carried tiles
            wstate = {"sb": None}

            def stageA(j):
                wcols = slice(j * NIWT, (j + 1) * NIWT)
                aiS = sp3.tile([P, ET], BF16, tag="aiS")
                nc.gpsimd.dma_gather(
                    aiS[:].unsqueeze(1), Tsrc[:, :], isrc[:, wcols], ET, ET, P,
                    transpose=True)
                aiD = sp3.tile([P, ET], BF16, tag="aiD")
                nc.gpsimd.dma_gather(
                    aiD[:].unsqueeze(1), Tsrc[:, :], idst[:, wcols], ET, ET, P,
                    transpose=True)
                oht = sp3.tile([P, ET], BF16, tag="oht")
                nc.sync.dma_start(oht[:], ohm_d[:, j * ET:(j + 1) * ET])
                # two transposes, 2 chunks each at 32-stride partition blocks
                btSs = []
                for g in range(2):
                    btp = pp.tile([64, P], F32, space="PSUM", tag="fr")
                    nc.tensor.transpose(
                        btp[:],
                        gfc[:, j * TILE_CH + 2 * g:j * TILE_CH + 2 * g + 2, :],
                        ident[:])
                    btS = sp.tile([64, P], BF16, tag=f"btS{g}")
                    nc.scalar.copy(btS[:], btp[:])
                    btSs.append(btS)
                # merged radial-l1 + sh selection: [18K -> 64 h1 | 44 shs]
                h1shp = pp.tile([108, ET], F32, space="PSUM", tag="fr")
                for cc in range(TILE_CH):
                    b = 32 * (cc % 2)
                    nc.tensor.matmul(h1shp[:, cc * P:(cc + 1) * P],
                                     W18[b:b + 18, :],
                                     btSs[cc // 2][b:b + 18, :],
                                     start=True, stop=True)
                h1 = sp.tile([64, ET], BF16, tag="eh1")
                nc.scalar.activation(h1[:], h1shp[0:64, :], AF.Silu)
                shs = shss[j % 4]
                nc.scalar.copy(shs[16:60, :], h1shp[64:108, :])
                h2p = pp.tile([64, ET], F32, space="PSUM", tag="fr")
                nc.tensor.matmul(h2p[:], W2e[:], h1[:], start=True, stop=True)
                h2 = sp.tile([64, ET], BF16, tag="eh2")
                nc.scalar.activation(h2[:], h2p[:], AF.Silu)
                h3p = pp.tile([64, ET], F32, space="PSUM", tag="fr")
                nc.tensor.matmul(h3p[:], W3e[:], h2[:], start=True, stop=True)
                w3b = sp.tile([64, ET], BF16, tag="ew3b")
                nc.scalar.activation(w3b[:], h3p[:], AF.Silu)
                st[j] = dict(aiS=aiS, aiD=aiD, oht=oht, w3b=w3b)

            def stageB(j):
                s = st[j]
                rqs = []
                for q in range(4):
                    wrp = pp.tile([P, ET], F32, space="PSUM", tag="wrp")
                    nc.tensor.matmul(wrp[:], Selt[:, 128 * q:128 * (q + 1)],
                                     s["w3b"][:], start=True, stop=True)
                    rq = sp.tile([P, ET], BF16, tag=f"rq{q}")
                    nc.vector.tensor_tensor(out=rq[:], in0=wrp[:],
                                            in1=s["aiD"][:], op=ALU.mult)
                    rqs.append(rq)
                tms = []
                for m in range(2):
                    cps = pp.tile([112, ET], F32, space="PSUM", tag="cps")
                    for q in range(4):
                        nc.tensor.matmul(cps[:],
                                         W4pt[:, q * 224 + m * 112:
                                              q * 224 + (m + 1) * 112],
                                         rqs[q][:], start=(q == 0), stop=(q == 3))
                    tm = sp.tile([112, ET], BF16, tag=f"tm{m}")
                    nc.vector.tensor_tensor(out=tm[:], in0=cps[:],
                                            in1=s["aiS"][0:112, :], op=ALU.mult)
                    tms.append(tm)
                s["tms"] = tms

            def stageC(j):
                s = st.pop(j)
                fps = pp.tile([60, ET], F32, space="PSUM", tag="ftp")
                nc.tensor.matmul(fps[:], Wbt[:], s["tms"][0][:],
                                 start=True, stop=False)
                nc.tensor.matmul(fps[:], Wat[:], s["tms"][1][:],
                                 start=False, stop=True)
                F = Fts[j % 2]
                nc.vector.tensor_tensor(out=F[0:60, :], in0=fps[:],
                                        in1=shss[j % 4][:], op=ALU.mult)
                segs = {}
                for cc in range(TILE_CH):
                    segs.setdefault((j * TILE_CH + cc) // W_CH, []).append(cc)
                for win, ccs in sorted(segs.items()):
                    fcs = []
                    for cc in ccs:
                        ftp = pp.tile([P, 64], BF16, space="PSUM", tag="ftp")
                        nc.tensor.transpose(ftp[:], F[:, cc * P:(cc + 1) * P],
                                            identb[0:64, 0:64])
                        fc = sp.tile([P, 60], BF16, tag="fc")
                        nc.scalar.copy(fc[:], ftp[:, 0:60])
                        fcs.append(fc)
                    wt = pp.tile([P, 60], F32, space="PSUM", tag="wrp")
                    for i, cc in enumerate(ccs):
                        nc.tensor.matmul(wt[:], s["oht"][:, cc * P:(cc + 1) * P],
                                         fcs[i][:], start=(i == 0),
                                         stop=(i == len(ccs) - 1))
                    first = (j * TILE_CH + ccs[0]) % W_CH == 0
                    last = (j * TILE_CH + ccs[-1]) % W_CH == W_CH - 1
                    if first:
                        wstate["sb"] = sp.tile([P, 60], F32, tag="winsb",
                                               name="win_sb")
                        nc.vector.tensor_copy(wstate["sb"][:], wt[:])
                    else:
                        nc.vector.tensor_tensor(out=wstate["sb"][:],
                                                in0=wstate["sb"][:],
                                                in1=wt[:], op=ALU.add)
                    if last:
                        nc.vector.tensor_tensor(
                            out=wstate["sb"][:], in0=wstate["sb"][:],
                            in1=rw[:, win:win + 1].to_broadcast([P, 60]),
                            op=ALU.mult)
                        nc.sync.dma_start(out_d[win * P:(win + 1) * P, :],
                                          wstate["sb"][:])

            for i in range(NT + 2):
                if i >= 2:
                    stageC(i - 2)
                if 1 <= i <= NT:
                    stageB(i - 1)
                if i < NT:
                    stageA(i)
    nc.compile()
    return nc


_CACHE = {}


def kernel(**inputs):
    per_core, onehotA, W_CH, C_TOT, E = _host_prep(inputs)
    et = np.asarray(inputs["embed_table"], np.float32)
    fW4 = np.asarray(inputs["fc_W4"], np.float32)
    consts = _build_consts(fW4)
    W4p, Sel, Wb, Wa, Selsh = consts
    key = (W_CH, C_TOT)
    if key not in _CACHE:
        _CACHE[key] = _build_bass(W_CH, C_TOT, E, consts)
    nc = _CACHE[key]
    W18 = np.zeros((128, 108), np.float32)
    for cc in range(4):
        W18[32 * cc:32 * cc + 10, 0:64] = \
            np.asarray(inputs["fc_W1"], np.float32) / 1.12
        W18[32 * cc + 10:32 * cc + 18, 64:108] = Selsh
    shared = dict(
        onehotA=onehotA,
        TA=(et @ np.asarray(inputs["fit_W1"], np.float32)).astype(np.float32),
        fit_W2=np.asarray(inputs["fit_W2"], np.float32),
        fit_W3=np.asarray(inputs["fit_W3"], np.float32),
        W18=W18,
        fc_W2p=(np.asarray(inputs["fc_W2"], np.float32) / 8.0),
        fc_W3p=(np.asarray(inputs["fc_W3"], np.float32) / 8.0),
        W4p=np.ascontiguousarray(np.transpose(W4p.reshape(4, 128, 224), (1, 0, 2)).reshape(128, 896)),
        cvec=np.tile(np.array([*(-VCENters / STEP), 1.0 / STEP, -1.0,
                               15.0 ** 0.5, 0.5 * 5.0 ** 0.5, 5.0 ** 0.5,
                               0.5 * 15.0 ** 0.5], np.float32), (P, 1)),
        Sel=np.ascontiguousarray(np.transpose(Sel, (1, 0, 2)).reshape(64, 512)),
        Wb=Wb, Wa=Wa,
    )
    import ml_dtypes
    for k in ("W4p", "Sel", "Wb", "Wa", "TA", "fit_W2", "fit_W3",
              "W18", "fc_W2p", "fc_W3p", "onehotA"):
        shared[k] = shared[k].astype(ml_dtypes.bfloat16)
    in_maps = []
    for ci in range(N_CORES):
        m = dict(shared)
        m.update(geom_pl=per_core[ci]["geom_pl"],
                 recip_win=per_core[ci]["recip_win"],
                 oh_pl=per_core[ci]["oh_pl"].astype(ml_dtypes.bfloat16),
                 idx_src=per_core[ci]["idx_src"], idx_dst=per_core[ci]["idx_dst"])
        in_maps.append(m)
    res = bass_utils.run_bass_kernel_spmd(nc, in_maps, core_ids=list(range(N_CORES)))
    out = np.concatenate([res.results[ci]["out"][:NPC] for ci in range(N_CORES)], 0)
    return out.astype(np.float32)


# revision 25
# speedup vs baseline: 1.2037x; 1.2037x over previous
"""E3Conv Trainium2 kernel: 8-core SPMD, dst-partitioned edges.

Strategy: sort edges by dst; core i owns nodes [1250i,1250(i+1)) and all edges
into them (no all-reduce needed). Per core: node-MLP replicated, one bf16
gather table for Ai (recip applied per-window at the end), radial MLP +
tensor-product restructured as one K=512 matmul per edge tile, PE-accumulated
one-hot matmul scatter. Engine-balanced: PSUM drains fused into DVE
tensor_tensor ops, copies/activations on ACT, scatter accumulation on PE.
"""
import sys, os
sys.path.insert(0, "/opt/trn_rl_repo")
import numpy as np

import concourse.bass as bass
import concourse.tile as tile
from concourse import bacc, mybir
from concourse import bass_utils
from concourse.masks import make_identity

P = 128
N_NODES, N_EDGES, N_GRAPHS = 10000, 131072, 64
N_CORES, NPC, N_WIN = 8, 1250, 10
MAX_RADIUS, N_BASIS = 4.0, 10
STEP = MAX_RADIUS / (N_BASIS + 1)
VCENters = np.linspace(0.0, MAX_RADIUS, N_BASIS + 2)[1:-1].astype(np.float32)
F32, BF16, F32R, I16 = (mybir.dt.float32, mybir.dt.bfloat16,
                        mybir.dt.float32r, mybir.dt.int16)
AF = mybir.ActivationFunctionType
ALU = mybir.AluOpType
NCH = 79  # node chunks of 128 (79*128 = 10112 >= 10000)
TILE_CH = 4
ET = TILE_CH * P  # 512


def _build_consts(fW4):
    s3 = 3.0 ** 0.5
    W4p = np.zeros((512, 224), np.float32)
    offs = {0: 0, 1: 1024, 2: 1536}
    wbase = {0: 0, 1: 16, 2: 24}
    scale_l = {0: 1.0 / 64, 1: s3 / 64, 2: 1.0 / 64}
    for l, mo in enumerate((16, 8, 4)):
        for u in range(8):
            for v in range(8):
                for wl in range(mo):
                    col = offs[l] + (u * 8 + v) * mo + wl
                    w = wbase[l] + wl
                    W4p[np.arange(64) * 8 + v, w * 8 + u] = fW4[:, col] * scale_l[l]
    Sel = np.zeros((4, 64, 128), np.float32)
    for q in range(4):
        for r in range(128):
            Sel[q, 16 * q + r // 8, r] = 1.0
    # L2 reduction weights -> single [112, 60] pair (chained over tm0/tm1)
    # output cols: 0:16 = l0 (16 w), 16:60 = 44 (l1 24 + l2 20) sh-multiplied
    L2_0 = np.zeros((112, 16), np.float32)
    for r in range(112):
        L2_0[r, r // 8] = 1.0
    L2_x = np.zeros((112, 16), np.float32)
    for r in range(16):
        L2_x[r, 14 + r // 8] = 1.0
    L2_1 = np.zeros((112, 44), np.float32)
    for r in range(112):
        w = 14 + r // 8
        if w < 16:
            pass
        elif w < 24:
            for m in range(3):
                L2_1[r, (w - 16) * 3 + m] = 1.0
        else:
            for m in range(5):
                L2_1[r, 24 + (w - 24) * 5 + m] = 1.0
    Wb = np.concatenate([L2_0, np.zeros((112, 44), np.float32)], 1)  # on tm0
    Wa = np.concatenate([L2_x, L2_1], 1)                             # on tm1
    Selsh = np.zeros((8, 44), np.float32)
    for w in range(8):
        for m in range(3):
            Selsh[m, w * 3 + m] = 1.0
    for w in range(4):
        for m in range(5):
            Selsh[3 + m, 24 + w * 5 + m] = 1.0
    return W4p, Sel, Wb, Wa, Selsh


def _host_prep(inputs):
    pos = np.asarray(inputs["pos"], np.float32)
    A = np.asarray(inputs["A"]).astype(np.int64)
    batch = np.asarray(inputs["batch"]).astype(np.int64)
    esrc = np.asarray(inputs["edge_src"]).astype(np.int64)
    edst = np.asarray(inputs["edge_dst"]).astype(np.int64)
    shifts = np.asarray(inputs["edge_shifts"], np.float32)
    cell = np.asarray(inputs["cell"], np.float32)
    counts = np.bincount(edst, minlength=N_NODES).astype(np.float32)
    recipc = 1.0 / np.maximum(counts, 1.0)
    cpn = cell[batch].reshape(N_NODES, 9)
    order = np.argsort(edst, kind="stable")
    wins_all, W_CH = [], 0
    for ci in range(N_CORES):
        lo = ci * NPC
        m = order[(edst[order] >= lo) & (edst[order] < lo + NPC)]
        wins = []
        for w in range(N_WIN):
            wlo = lo + w * P
            whi = min(lo + (w + 1) * P, lo + NPC)
            wm = m[(edst[m] >= wlo) & (edst[m] < whi)]
            wins.append(wm)
            W_CH = max(W_CH, (len(wm) + P - 1) // P)
        wins_all.append(wins)
    while (N_WIN * W_CH) % TILE_CH:
        W_CH += 1
    C_TOT = N_WIN * W_CH
    E = C_TOT * P
    onehotA = np.zeros((10, N_NODES), np.float32)
    onehotA[A, np.arange(N_NODES)] = 1.0
    per_core = []
    for ci in range(N_CORES):
        idx = np.zeros(E, np.int64)
        valid = np.zeros(E, bool)
        dstloc = np.full(E, -1.0, np.float32)
        for w in range(N_WIN):
            wm = wins_all[ci][w]
            s = w * W_CH * P
            idx[s:s + len(wm)] = wm
            valid[s:s + len(wm)] = True
            dstloc[s:s + len(wm)] = (edst[wm] - ci * NPC - w * P).astype(np.float32)
        src = np.where(valid, esrc[idx], 0)
        dst = np.where(valid, edst[idx], 0)
        sh = np.where(valid[:, None], shifts[idx], np.float32(1.0))
        geom = np.concatenate([pos[src], pos[dst], sh, cpn[src]], 1)  # [E,18]
        geom_pl = np.ascontiguousarray(
            np.transpose(geom.reshape(C_TOT, P, 18), (1, 2, 0)).reshape(P, 18 * C_TOT))

        def wrap(ix):
            wr = ix.astype(np.int16).reshape(-1, 16).T  # [16, E/16]
            return np.ascontiguousarray(np.tile(wr, (8, 1)))
        ohm = (dstloc.reshape(C_TOT, P, 1) ==
               np.arange(P, dtype=np.float32)[None, None, :])
        oh_pl = np.ascontiguousarray(
            np.transpose(ohm, (1, 0, 2)).reshape(P, C_TOT * P))
        rw = np.ones((P, N_WIN), np.float32)
        for w in range(N_WIN):
            g0 = ci * NPC + w * P
            n = min(P, NPC - w * P)
            rw[:n, w] = recipc[g0:g0 + n]
        per_core.append(dict(geom_pl=geom_pl, oh_pl=oh_pl, recip_win=rw,
                             idx_src=wrap(src), idx_dst=wrap(dst)))
    return per_core, onehotA, W_CH, C_TOT, E


def _build_bass(W_CH, C_TOT, E, consts):
    W4p, Sel, Wb, Wa, Selsh = consts
    NT = C_TOT // TILE_CH
    NIW = E // 16
    NIWT = NIW // NT
    nc = bacc.Bacc("TRN2", target_bir_lowering=False, debug=False,
                   num_devices=N_CORES)

    def din(name, shape, dt=F32):
        return nc.dram_tensor(name, shape, dt, kind="ExternalInput").ap()

    geom_d = din("geom_pl", [P, 18 * C_TOT])
    ohm_d = din("oh_pl", [P, C_TOT * P], BF16)
    isrc_d = din("idx_src", [P, NIW], I16)
    idst_d = din("idx_dst", [P, NIW], I16)
    ohA_d = din("onehotA", [10, N_NODES], BF16)
    rw_d = din("recip_win", [P, N_WIN])
    TA_d = din("TA", [10, 64], BF16)
    W2_d = din("fit_W2", [64, 32], BF16)
    W3_d = din("fit_W3", [32, 8], BF16)
    W18_d = din("W18", [128, 108], BF16)
    fW2_d = din("fc_W2p", [64, 64], BF16)
    fW3_d = din("fc_W3p", [64, 64], BF16)
    W4p_d = din("W4p", [128, 4 * 224], BF16)
    Sel_d = din("Sel", [64, 4 * 128], BF16)
    Wb_d = din("Wb", [112, 60], BF16)
    Wa_d = din("Wa", [112, 60], BF16)
    cv_d = din("cvec", [P, 16])
    out_d = nc.dram_tensor("out", [N_WIN * P, 60], F32, kind="ExternalOutput").ap()

    C = C_TOT
    with tile.TileContext(nc) as tc:
        with tc.tile_pool(name="const", bufs=1) as cp, \
             tc.tile_pool(name="sb", bufs=2) as sp, \
             tc.tile_pool(name="sb3", bufs=3) as sp3, \
             tc.tile_pool(name="sb4", bufs=4) as sp4, \
             tc.tile_pool(name="big", bufs=1) as bp, \
             tc.tile_pool(name="ps", bufs=2, space="PSUM") as pp, \
             tc.tile_pool(name="dram", bufs=1, space="DRAM") as dp:
            ident = cp.tile([P, P], F32)
            make_identity(nc, ident[:])
            identb = cp.tile([P, P], BF16)
            nc.vector.tensor_copy(identb[:], ident[:])

            def load_const(dram, shape, dt=F32):
                t = cp.tile(shape, dt, tag=dram.tensor.name)
                nc.sync.dma_start(t[:], dram[:])
                return t
            TA = load_const(TA_d, [10, 64], BF16)
            W2 = load_const(W2_d, [64, 32], BF16)
            W3 = load_const(W3_d, [32, 8], BF16)
            W18 = load_const(W18_d, [128, 108], BF16)
            W2e = load_const(fW2_d, [64, 64], BF16)
            W3e = load_const(fW3_d, [64, 64], BF16)
            W4pt = load_const(W4p_d, [128, 4 * 224], BF16)
            Selt = load_const(Sel_d, [64, 4 * 128], BF16)
            Wbt = load_const(Wb_d, [112, 60], BF16)
            Wat = load_const(Wa_d, [112, 60], BF16)
            cv = load_const(cv_d, [P, 16])
            rw = load_const(rw_d, [P, N_WIN])
            ohA = bp.tile([10, N_NODES], BF16)
            nc.sync.dma_start(ohA[:], ohA_d[:])
            isrc = bp.tile([P, NIW], I16)
            nc.sync.dma_start(isrc[:], isrc_d[:])
            idst = bp.tile([P, NIW], I16)
            nc.sync.dma_start(idst[:], idst_d[:])

            # ---------------- node MLP -> gather table ----------------
            Tsrc = dp.tile([NCH * P, P], BF16)
            Ai_sb = bp.tile([8, NCH * P], F32)
            nc.gpsimd.memset(Ai_sb[:, N_NODES:], 0.0)
            for j in range(20):
                s = j * 512
                n = min(512, N_NODES - s)
                h1p = pp.tile([64, 512], F32, space="PSUM", tag="fr")
                nc.tensor.matmul(h1p[:, :n], TA[:], ohA[:, s:s + n],
                                 start=True, stop=True)
                h1 = sp.tile([64, 512], BF16, tag="h1n")
                nc.scalar.activation(h1[:, :n], h1p[:, :n], AF.Silu)
                h2p = pp.tile([32, 512], F32, space="PSUM", tag="fr")
                nc.tensor.matmul(h2p[:, :n], W2[:], h1[:, :n],
                                 start=True, stop=True)
                h2 = sp.tile([32, 512], BF16, tag="h2n")
                nc.scalar.activation(h2[:, :n], h2p[:, :n], AF.Silu)
                aip = pp.tile([8, 512], F32, space="PSUM", tag="fr")
                nc.tensor.matmul(aip[:, :n], W3[:], h2[:, :n],
                                 start=True, stop=True)
                nc.vector.tensor_copy(Ai_sb[:, s:s + n], aip[:, :n])
            # grouped table build: 8 node-chunks per round to amortize latency
            for g0 in range(0, NCH, 8):
                gk = min(8, NCH - g0)
                tpp = pp.tile([P, 64], F32, space="PSUM", tag="fr")
                for c in range(gk):
                    nc.tensor.transpose(tpp[:, 8 * c:8 * c + 8],
                                        Ai_sb[:, (g0 + c) * P:(g0 + c + 1) * P],
                                        ident[0:8, 0:8])
                f16g = sp.tile([P, 64], BF16, tag="f16g")
                nc.scalar.copy(f16g[:, :8 * gk], tpp[:, :8 * gk])
                repg = sp.tile([P, 8 * P], BF16, tag="repg")
                nc.vector.tensor_copy(
                    repg[:].rearrange("p (k r v) -> p k r v", k=8, v=8)
                        [:, 0:gk],
                    f16g[:].rearrange("p (k v) -> p k v", k=8)
                        .unsqueeze(2).to_broadcast([P, 8, 16, 8])[:, 0:gk])
                nc.sync.dma_start(
                    Tsrc[g0 * P:(g0 + gk) * P, :]
                        .rearrange("(k p) c -> p k c", k=gk),
                    repg[:].rearrange("p (k c) -> p k c", k=8)[:, 0:gk])

            # ---------------- geometry (plane layout, whole E) ----------------
            gm = bp.tile([P, 18 * C], F32)
            nc.sync.dma_start(gm[:], geom_d[:])
            g3 = gm[:].rearrange("p (f c) -> p f c", f=18)
            tmp9 = bp.tile([P, 9 * C], F32)
            nc.vector.tensor_tensor(
                out=tmp9[:].rearrange("p (i j c) -> p i j c", i=3, j=3),
                in0=gm[:, 9 * C:18 * C].rearrange("p (i j c) -> p i j c", i=3, j=3),
                in1=gm[:, 6 * C:9 * C].rearrange("p (i c) -> p i c", i=3)
                    .unsqueeze(2).to_broadcast([P, 3, 3, C]),
                op=ALU.mult)
            sv = bp.tile([P, 3 * C], F32)
            nc.vector.tensor_tensor(out=sv[:], in0=tmp9[:, 0:3 * C],
                                    in1=tmp9[:, 3 * C:6 * C], op=ALU.add)
            nc.vector.tensor_tensor(out=sv[:], in0=sv[:],
                                    in1=tmp9[:, 6 * C:9 * C], op=ALU.add)
            ev = bp.tile([P, 3 * C], F32)
            nc.vector.tensor_tensor(out=ev[:], in0=g3[:, 3:6].rearrange("p f c -> p (f c)"),
                                    in1=g3[:, 0:3].rearrange("p f c -> p (f c)"),
                                    op=ALU.subtract)
            nc.vector.tensor_tensor(out=ev[:], in0=ev[:], in1=sv[:], op=ALU.add)
            sq = bp.tile([P, 3 * C], F32)
            nc.vector.tensor_tensor(out=sq[:], in0=ev[:], in1=ev[:], op=ALU.mult)
            ln2 = bp.tile([P, C], F32)
            nc.vector.tensor_tensor(out=ln2[:], in0=sq[:, 0:C], in1=sq[:, C:2 * C],
                                    op=ALU.add)
            nc.vector.tensor_tensor(out=ln2[:], in0=ln2[:], in1=sq[:, 2 * C:3 * C],
                                    op=ALU.add)
            ln = bp.tile([P, C], F32)
            nc.scalar.activation(ln[:], ln2[:], AF.Sqrt)
            rl = bp.tile([P, C], F32)
            nc.vector.reciprocal(rl[:], ln[:])
            u = bp.tile([P, 3 * C], F32)
            nc.vector.tensor_tensor(
                out=u[:].rearrange("p (f c) -> p f c", f=3),
                in0=ev[:].rearrange("p (f c) -> p f c", f=3),
                in1=rl[:].unsqueeze(1).to_broadcast([P, 3, C]), op=ALU.mult)
            usq = bp.tile([P, 3 * C], F32)
            nc.vector.tensor_tensor(out=usq[:], in0=u[:], in1=u[:], op=ALU.mult)
            # feature planes tile: f-major [basis10 | sh1 3 | sh2 5]
            gf = bp.tile([P, 32 * C], F32)
            nc.gpsimd.memset(gf[:, 18 * C:32 * C], 0.0)
            dt2 = bp.tile([P, 10 * C], F32)
            for b in range(N_BASIS):
                nc.scalar.activation(dt2[:, b * C:(b + 1) * C], ln[:], AF.Square,
                                     bias=cv[:, b:b + 1],
                                     scale=cv[:, 10:11])
            nc.scalar.activation(gf[:, 0:10 * C], dt2[:], AF.Exp,
                                 scale=cv[:, 11:12])
            nc.vector.tensor_copy(gf[:, 10 * C:13 * C], u[:])
            t1 = bp.tile([P, C], F32)
            nc.scalar.mul(t1[:], u[:, 2 * C:3 * C], cv[:, 12:13])       # sqrt15*uz
            nc.vector.tensor_tensor(out=gf[:, 13 * C:14 * C], in0=u[:, 0:C],
                                    in1=t1[:], op=ALU.mult)     # m0
            nc.vector.tensor_tensor(out=gf[:, 16 * C:17 * C], in0=u[:, C:2 * C],
                                    in1=t1[:], op=ALU.mult)     # m3
            nc.scalar.mul(t1[:], u[:, 0:C], cv[:, 12:13])               # sqrt15*ux
            nc.vector.tensor_tensor(out=gf[:, 14 * C:15 * C], in0=u[:, C:2 * C],
                                    in1=t1[:], op=ALU.mult)     # m1
            t2 = bp.tile([P, C], F32)
            nc.vector.tensor_tensor(out=t2[:], in0=usq[:, 0:C],
                                    in1=usq[:, 2 * C:3 * C], op=ALU.add)
            nc.scalar.mul(t2[:], t2[:], cv[:, 13:14])
            t3 = bp.tile([P, C], F32)
            nc.scalar.mul(t3[:], usq[:, C:2 * C], cv[:, 14:15])
            nc.vector.tensor_tensor(out=gf[:, 15 * C:16 * C], in0=t3[:], in1=t2[:],
                                    op=ALU.subtract)            # m2
            nc.vector.tensor_tensor(out=t2[:], in0=usq[:, 2 * C:3 * C],
                                    in1=usq[:, 0:C], op=ALU.subtract)
            nc.scalar.mul(gf[:, 17 * C:18 * C], t2[:], cv[:, 15:16])  # m4
            gfc = gf[:].rearrange("p (f c) -> p c f", f=32)

            # F tiles: rows 60:64 must stay zero (read by the transpose);
            # shs tiles: rows 0:16 stay 1.0 (l0 passthrough in the F multiply)
            Fts, shss = [], []
            for fi in range(2):
                Ft = sp.tile([64, ET], BF16, tag="F", name=f"Ft{fi}")
                nc.gpsimd.memset(Ft[:], 0.0)
                Fts.append(Ft)
            for fi in range(4):
                sht = sp4.tile([60, ET], BF16, tag="shs", name=f"sht{fi}")
                nc.gpsimd.memset(sht[0:16, :], 1.0)
                shss.append(sht)

            # ---------------- edge tiles: 3-stage software pipeline ------
            # stage A(j): gathers + geometry transpose + radial MLP front
            # stage B(j): Sel expansion, aiD-mult, K=512 contraction, aiS-mult
            # stage C(j): L2 reduction, sh-mult, transpose + one-hot scatter
            st = {}      # per-tile carried tiles
            wstate = {"sb": None}

            def stageA(j):
                wcols = slice(j * NIWT, (j + 1) * NIWT)
                aiS = sp3.tile([P, ET], BF16, tag="aiS")
                nc.gpsimd.dma_gather(
                    aiS[:].unsqueeze(1), Tsrc[:, :], isrc[:, wcols], ET, ET, P,
                    transpose=True)
                aiD = sp3.tile([P, ET], BF16, tag="aiD")
                nc.gpsimd.dma_gather(
                    aiD[:].unsqueeze(1), Tsrc[:, :], idst[:, wcols], ET, ET, P,
                    transpose=True)
                oht = sp3.tile([P, ET], BF16, tag="oht")
                nc.sync.dma_start(oht[:], ohm_d[:, j * ET:(j + 1) * ET])
                # two transposes, 2 chunks each at 32-stride partition blocks
                btSs = []
                for g in range(2):
                    btp = pp.tile([64, P], F32, space="PSUM", tag="fr")
                    nc.tensor.transpose(
                        btp[:],
                        gfc[:, j * TILE_CH + 2 * g:j * TILE_CH + 2 * g + 2, :],
                        ident[:])
                    btS = sp.tile([64, P], BF16, tag=f"btS{g}")
                    nc.scalar.copy(btS[:], btp[:])
                    btSs.append(btS)
                # merged radial-l1 + sh selection: [18K -> 64 h1 | 44 shs]
                h1shp = pp.tile([108, ET], F32, space="PSUM", tag="fr")
                for cc in range(TILE_CH):
                    b = 32 * (cc % 2)
                    nc.tensor.matmul(h1shp[:, cc * P:(cc + 1) * P],
                                     W18[b:b + 18, :],
                                     btSs[cc // 2][b:b + 18, :],
                                     start=True, stop=True)
                h1 = sp.tile([64, ET], BF16, tag="eh1")
                nc.scalar.activation(h1[:], h1shp[0:64, :], AF.Silu)
                shs = shss[j % 4]
                nc.scalar.copy(shs[16:60, :], h1shp[64:108, :])
                h2p = pp.tile([64, ET], F32, space="PSUM", tag="fr")
                nc.tensor.matmul(h2p[:], W2e[:], h1[:], start=True, stop=True)
                h2 = sp.tile([64, ET], BF16, tag="eh2")
                nc.scalar.activation(h2[:], h2p[:], AF.Silu)
                h3p = pp.tile([64, ET], F32, space="PSUM", tag="fr")
                nc.tensor.matmul(h3p[:], W3e[:], h2[:], start=True, stop=True)
                w3b = sp.tile([64, ET], BF16, tag="ew3b")
                nc.scalar.activation(w3b[:], h3p[:], AF.Silu)
                st[j] = dict(aiS=aiS, aiD=aiD, oht=oht, w3b=w3b)

            def stageB(j):
                s = st[j]
                rqs = []
                for q in range(4):
                    wrp = pp.tile([P, ET], F32, space="PSUM", tag="wrp")
                    nc.tensor.matmul(wrp[:], Selt[:, 128 * q:128 * (q + 1)],
                                     s["w3b"][:], start=True, stop=True)
                    rq = sp.tile([P, ET], BF16, tag=f"rq{q}")
                    nc.vector.tensor_tensor(out=rq[:], in0=wrp[:],
                                            in1=s["aiD"][:], op=ALU.mult)
                    rqs.append(rq)
                tms = []
                for m in range(2):
                    cps = pp.tile([112, ET], F32, space="PSUM", tag="cps")
                    for q in range(4):
                        nc.tensor.matmul(cps[:],
                                         W4pt[:, q * 224 + m * 112:
                                              q * 224 + (m + 1) * 112],
                                         rqs[q][:], start=(q == 0), stop=(q == 3))
                    tm = sp.tile([112, ET], BF16, tag=f"tm{m}")
                    nc.vector.tensor_tensor(out=tm[:], in0=cps[:],
                                            in1=s["aiS"][0:112, :], op=ALU.mult)
                    tms.append(tm)
                s["tms"] = tms

            def stageC(j):
                s = st.pop(j)
                fps = pp.tile([60, ET], F32, space="PSUM", tag="ftp")
                nc.tensor.matmul(fps[:], Wbt[:], s["tms"][0][:],
                                 start=True, stop=False)
                nc.tensor.matmul(fps[:], Wat[:], s["tms"][1][:],
                                 start=False, stop=True)
                F = Fts[j % 2]
                nc.vector.tensor_tensor(out=F[0:60, :], in0=fps[:],
                                        in1=shss[j % 4][:], op=ALU.mult)
                segs = {}
                for cc in range(TILE_CH):
                    segs.setdefault((j * TILE_CH + cc) // W_CH, []).append(cc)
                for win, ccs in sorted(segs.items()):
                    fcs = []
                    for cc in ccs:
                        ftp = pp.tile([P, 64], BF16, space="PSUM", tag="ftp")
                        nc.tensor.transpose(ftp[:], F[:, cc * P:(cc + 1) * P],
                                            identb[0:64, 0:64])
                        fc = sp.tile([P, 60], BF16, tag="fc")
                        nc.scalar.copy(fc[:], ftp[:, 0:60])
                        fcs.append(fc)
                    wt = pp.tile([P, 60], F32, space="PSUM", tag="wrp")
                    for i, cc in enumerate(ccs):
                        nc.tensor.matmul(wt[:], s["oht"][:, cc * P:(cc + 1) * P],
                                         fcs[i][:], start=(i == 0),
                                         stop=(i == len(ccs) - 1))
                    first = (j * TILE_CH + ccs[0]) % W_CH == 0
                    last = (j * TILE_CH + ccs[-1]) % W_CH == W_CH - 1
                    if first:
                        wstate["sb"] = sp.tile([P, 60], F32, tag="winsb",
                                               name="win_sb")
                        nc.vector.tensor_copy(wstate["sb"][:], wt[:])
                    else:
                        nc.vector.tensor_tensor(out=wstate["sb"][:],
                                                in0=wstate["sb"][:],
                                                in1=wt[:], op=ALU.add)
                    if last:
                        nc.vector.tensor_tensor(
                            out=wstate["sb"][:], in0=wstate["sb"][:],
                            in1=rw[:, win:win + 1].to_broadcast([P, 60]),
                            op=ALU.mult)
                        nc.sync.dma_start(out_d[win * P:(win + 1) * P, :],
                                          wstate["sb"][:])

            for i in range(NT + 2):
                if i >= 2:
                    stageC(i - 2)
                if 1 <= i <= NT:
                    stageB(i - 1)
                if i < NT:
                    stageA(i)
    nc.compile()
    return nc


_CACHE = {}


def kernel(**inputs):
    per_core, onehotA, W_CH, C_TOT, E = _host_prep(inputs)
    et = np.asarray(inputs["embed_table"], np.float32)
    fW4 = np.asarray(inputs["fc_W4"], np.float32)
    consts = _build_consts(fW4)
    W4p, Sel, Wb, Wa, Selsh = consts
    key = (W_CH, C_TOT)
    if key not in _CACHE:
        _CACHE[key] = _build_bass(W_CH, C_TOT, E, consts)
    nc = _CACHE[key]
    W18 = np.zeros((128, 108), np.float32)
    for cc in range(4):
        W18[32 * cc:32 * cc + 10, 0:64] = \
            np.asarray(inputs["fc_W1"], np.float32) / 1.12
        W18[32 * cc + 10:32 * cc + 18, 64:108] = Selsh
    shared = dict(
        onehotA=onehotA,
        TA=(et @ np.asarray(inputs["fit_W1"], np.float32)).astype(np.float32),
        fit_W2=np.asarray(inputs["fit_W2"], np.float32),
        fit_W3=np.asarray(inputs["fit_W3"], np.float32),
        W18=W18,
        fc_W2p=(np.asarray(inputs["fc_W2"], np.float32) / 8.0),
        fc_W3p=(np.asarray(inputs["fc_W3"], np.float32) / 8.0),
        W4p=np.ascontiguousarray(np.transpose(W4p.reshape(4, 128, 224), (1, 0, 2)).reshape(128, 896)),
        cvec=np.tile(np.array([*(-VCENters / STEP), 1.0 / STEP, -1.0,
                               15.0 ** 0.5, 0.5 * 5.0 ** 0.5, 5.0 ** 0.5,
                               0.5 * 15.0 ** 0.5], np.float32), (P, 1)),
        Sel=np.ascontiguousarray(np.transpose(Sel, (1, 0, 2)).reshape(64, 512)),
        Wb=Wb, Wa=Wa,
    )
    import ml_dtypes
    for k in ("W4p", "Sel", "Wb", "Wa", "TA", "fit_W2", "fit_W3",
              "W18", "fc_W2p", "fc_W3p", "onehotA"):
        shared[k] = shared[k].astype(ml_dtypes.bfloat16)
    in_maps = []
    for ci in range(N_CORES):
        m = dict(shared)
        m.update(geom_pl=per_core[ci]["geom_pl"],
                 recip_win=per_core[ci]["recip_win"],
                 oh_pl=per_core[ci]["oh_pl"].astype(ml_dtypes.bfloat16),
                 idx_src=per_core[ci]["idx_src"], idx_dst=per_core[ci]["idx_dst"])
        in_maps.append(m)
    res = bass_utils.run_bass_kernel_spmd(nc, in_maps, core_ids=list(range(N_CORES)))
    out = np.concatenate([res.results[ci]["out"][:NPC] for ci in range(N_CORES)], 0)
    return out.astype(np.float32)
